# revision 12
# baseline (speedup 1.0000x reference)
"""Trainium2 Bass kernel for nn_CriticNetwork (LSTM T=3, D=18, H=64 + MLP 64->32->1).

v2 — engine-balanced redesign of the baseline:
  * All gate activations are Sigmoid on ScalarE (tanh(g) = 2*sigmoid(2g)-1,
    the 2x folded into the g-gate weights at host prep; the (2s-1) affine is
    fused into the IG custom DVE op).  One sigmoid instr per step over the
    packed [i|o|g|f] PSUM tile (FD 1536 for step 1).
  * h = sigmoid(o) * tanh(c) fused into one custom DVE op (TH7MUL, deg-7
    odd poly with leading coeff pinned to 1 so only 3 scalar slots needed).
  * f*c and c-adds run on GpSimd; customs + relu on VectorE.
  * GpSimd partition-shift h-stores replaced by SBUF->SBUF DMA (block-half
    batched, strided 3D APs) into the K-stacked rhs tiles.
  * MLP: W1 augmented with a 33rd constant row so b2 folds into W2 (no
    bias/identity instr); value PSUM copied to SBUF once per iter, output
    DMA'd twice per block.
  * Step-1 matmuls 6-way tile_position packed (rows 0/32/64 x cols 0/64)
    with x triplicated across partition strips; PE warm-up burst up front.

Layout: feature-rows on partitions, batch on the free dim; per core 64
iterations of 1024 elements (A/B 512-wide halves on partition rows 0:64 /
64:128), phased per 8-iteration block.
"""
import os
import numpy as np
import ml_dtypes

import concourse.bacc as bacc
import concourse.bass as bass
import concourse.mybir as mybir
import concourse.tile as tile
from concourse import bass_utils

F32 = mybir.dt.float32
BF16 = mybir.dt.bfloat16
AF = mybir.ActivationFunctionType
ALU = mybir.AluOpType

NCORES = 8
W = 512                        # sub-tile width (psum bank)
ITERS = int(os.environ.get("K_ITERS", "64"))
BCORE = ITERS * 2 * W          # 65536 at full size
BATCH = BCORE * NCORES
BLK = int(os.environ.get("K_BLK", "8"))  # iters per phase/DMA block
NBLK = ITERS // BLK

STATE_DIM, SEQ_LEN, HIDDEN, MLP_HIDDEN = 18, 3, 64, 32
KX = STATE_DIM + 2             # x rows + two ones rows (bias hi/lo)
KS = HIDDEN + KX               # 84: [h; x; ones]

# tanh deg-5 odd poly, leading coeff pinned to 1 (for fused h = o*tanh(c)
# in steps 1-2; the error attenuates through later steps):
# tanh(x) ~ x*(1 + u*(C3 + u*C5)), u=x^2, |x|<=1.6, maxerr 1.2e-2
T5C3, T5C5 = -0.280228505, 0.0459361381
# tanh deg-7 odd poly on [-1.5, 1.5], maxerr 5.1e-4 (step 3, exact path)
T7C = (0.9967175625159229, -0.3102624127429846,
       0.08661915494425512, -0.011767701262857437)

# free-dim slot (cols) per gate in the packed PSUM tile [i|o|g|f]
SLOT = {"i": 0, "o": W, "g": 2 * W, "f": 3 * W}
# weight-matrix column base per gate in the packed wks tile
WCOL = {"i": 0, "o": 64, "g": 128, "f": 192}

_ops = {}


def get_custom_ops():
    """Register IG / TH5MUL / TANH7C custom DVE ops (idempotent)."""
    if _ops:
        return _ops["IG"], _ops["TH5MUL"], _ops["T7"]
    import concourse.dve_ops as dve_ops
    from concourse.dve_spec import (Spec, Src0, Src1, C0, C1, C2, C3, One, sq,
                                    lower, _spill_c3_to_src1)
    from concourse.dve_uop import DveOpSpec

    def register(name, body, ref):
        for op in dve_ops.OPS:
            if op.name == name:
                return op
        spec = Spec(body=body, reference=ref)
        if name not in dve_ops._SUB_OPCODE_FOR_NAME:
            dve_ops._SUB_OPCODE_FOR_NAME[name] = (
                max(dve_ops._SUB_OPCODE_FOR_NAME.values()) + 1)
        shas = {}
        for ver in ("v3", "v4"):
            try:
                s = DveOpSpec(name=name, opcode=dve_ops.get_dve_sub_opcode(name),
                              uops=lower(spec, ver=ver), rd1_en=True)
                shas[ver] = s.sha(ver)
            except Exception:
                pass
        op = dve_ops.DveOp(name, spec, subdim=False, uops_sha=shas)
        dve_ops.OPS.append(op)
        return op

    # IG: out = in0 * (2*in1 - 1)   (sigmoid(i) * tanh-from-sigmoid(g))
    def _ig_ref(in0, in1, s0, s1, imm2):
        return in0.astype(np.float32) * (2.0 * in1.astype(np.float32) - 1.0)

    ig = register("IG_SIGTANH_ANT", Src0 * (Src1 + Src1 - One), _ig_ref)

    # TH5MUL: out = in1 * tanh5(in0),
    #   tanh5(z) = z*(1 + u*(C0 + u*C1)), u = z^2   (7 ALU stages)
    u = sq(Src0)
    th_body = ((C1 * u + C0) * u + One) * Src0 * Src1

    def _th_ref(in0, in1, s0, s1, imm2):
        z = in0.astype(np.float32)
        uu = z * z
        t = ((s1 * uu + s0) * uu + 1.0) * z
        return t * in1.astype(np.float32)

    th = register("TH5MUL_ANT", th_body, _th_ref)

    # TANH7C: out = tanh7(in0), deg-7 odd, C3 spilled to a [P,1] in1 column
    u7 = sq(Src0)
    t7_body = _spill_c3_to_src1((((C3 * u7 + C2) * u7 + C1) * u7 + C0) * Src0)

    def _t7_ref(in0, in1, s0, s1, imm2):
        uu = in0.astype(np.float32) ** 2
        c3 = np.asarray(in1, np.float32).reshape(in1.shape[0], 1)
        return ((((c3 * uu + imm2) * uu + s1) * uu) + s0) * in0

    t7 = register("TANH7C_ANT", t7_body, _t7_ref)
    _ops["IG"], _ops["TH5MUL"], _ops["T7"] = ig, th, t7
    return ig, th, t7


def build_bass():
    ig_op, th5_op, t7_op = get_custom_ops()
    nc = bacc.Bacc("TRN2", target_bir_lowering=False, debug=False)

    xt0_d = nc.dram_tensor("xt0", [KX, BCORE], BF16, kind="ExternalInput").ap()
    xt1_d = nc.dram_tensor("xt1", [KX, BCORE], BF16, kind="ExternalInput").ap()
    xt2_d = nc.dram_tensor("xt2", [KX, BCORE], BF16, kind="ExternalInput").ap()
    wk0_d = nc.dram_tensor("wk0s", [KS, 64], BF16, kind="ExternalInput").ap()
    wks_d = nc.dram_tensor("wks", [KS, 256], BF16, kind="ExternalInput").ap()
    w1_d = nc.dram_tensor("w1r", [128, 33], BF16, kind="ExternalInput").ap()
    w2_d = nc.dram_tensor("w2r", [97, 1], BF16, kind="ExternalInput").ap()
    b1_d = nc.dram_tensor("b1r", [97, 1], F32, kind="ExternalInput").ap()
    out_d = nc.dram_tensor("out", [2, ITERS, W], F32, kind="ExternalOutput").ap()

    BW = BLK * 2 * W      # rhs block width (8192)
    HW_ = BLK * W         # h block width (4096)

    with tile.TileContext(nc) as tc:
        with tc.tile_pool(name="const", bufs=1) as cpool, \
             tc.tile_pool(name="xt0", bufs=2) as xt0p, \
             tc.tile_pool(name="s1", bufs=2) as s1p, \
             tc.tile_pool(name="s2", bufs=2) as s2p, \
             tc.tile_pool(name="sg", bufs=5) as sgp, \
             tc.tile_pool(name="cw", bufs=BLK + 6) as cwp, \
             tc.tile_pool(name="uv", bufs=6) as uvp, \
             tc.tile_pool(name="hh", bufs=2) as hhp, \
             tc.tile_pool(name="h3", bufs=4) as h3p, \
             tc.tile_pool(name="zr", bufs=4) as zrp, \
             tc.tile_pool(name="vo", bufs=1) as vop, \
             tc.tile_pool(name="pg", bufs=2, space="PSUM") as pgp:

            wk0s = cpool.tile([KS, 64], BF16)
            nc.sync.dma_start(wk0s[:], wk0_d[:])
            wks = cpool.tile([KS, 256], BF16)
            nc.sync.dma_start(wks[:], wks_d[:])
            w1r = cpool.tile([128, 33], BF16)
            nc.sync.dma_start(w1r[:], w1_d[:])
            w2r = cpool.tile([97, 1], BF16)
            nc.sync.dma_start(w2r[:], w2_d[:])
            b1r = cpool.tile([97, 1], F32)
            nc.sync.dma_start(b1r[:], b1_d[:])
            wsrc = cpool.tile([KS, W], BF16)
            nc.vector.memset(wsrc[:], 0.01)
            c3t = cpool.tile([128, 1], F32)
            nc.vector.memset(c3t[:], T7C[3])

            # PE warm-up burst: ~16 back-to-back matmuls (>3.4us busy)
            wpt = pgp.tile([128, 2048], F32, tag="g")
            for k in range(16):
                nc.tensor.matmul(wpt[0:64, (k % 4) * W:(k % 4 + 1) * W],
                                 wks[0:KS, 0:64], wsrc[0:KS, :],
                                 start=True, stop=True, tile_position=(0, 0),
                                 skip_group_check=True)

            def gate_mms(wk, kk, rhs_blk, cA, G, gates):
                for gn in gates:
                    lhs = wk[0:kk, WCOL[gn]:WCOL[gn] + 64]
                    s = SLOT[gn]
                    nc.tensor.matmul(G[0:64, s:s + W], lhs,
                                     rhs_blk[0:kk, cA:cA + W],
                                     start=True, stop=True, tile_position=(0, 0),
                                     skip_group_check=True)
                    nc.tensor.matmul(G[64:128, s:s + W], lhs,
                                     rhs_blk[0:kk, cA + W:cA + 2 * W],
                                     start=True, stop=True, tile_position=(0, 64),
                                     skip_group_check=True)

            for blk in range(NBLK):
                bw0 = blk * BW
                xt0b = xt0p.tile([KS, BW], BF16)
                for r0 in (0, 32, 64):
                    nc.sync.dma_start(xt0b[r0:r0 + KX, :],
                                      xt0_d[:, bw0:bw0 + BW])
                s1b = s1p.tile([KS, BW], BF16)
                nc.sync.dma_start(s1b[HIDDEN:KS, :], xt1_d[:, bw0:bw0 + BW])
                s2b = s2p.tile([KS, BW], BF16)
                nc.sync.dma_start(s2b[HIDDEN:KS, :], xt2_d[:, bw0:bw0 + BW])
                H1 = hhp.tile([128, HW_], BF16, tag="H1")
                H2 = hhp.tile([128, HW_], BF16, tag="H2")
                vo = vop.tile([33, HW_], F32, tag="vo")

                def shift_h(H, sb, half):
                    # scatter h block-half into the K-stack rhs rows 0:64
                    j0 = half * (BLK // 2)
                    n = BLK // 2
                    src_a = H[0:64, j0 * W:(j0 + n) * W].rearrange(
                        "p (a b) -> p a b", b=W)
                    src_b = H[64:128, j0 * W:(j0 + n) * W].rearrange(
                        "p (a b) -> p a b", b=W)
                    dst = sb[0:HIDDEN, j0 * 2 * W:(j0 + n) * 2 * W].rearrange(
                        "p (a b) -> p a b", b=2 * W)
                    nc.sync.dma_start(dst[:, :, 0:W], src_a)
                    nc.sync.dma_start(dst[:, :, W:2 * W], src_b)

                cs = {}
                # ---- phase 1: step 1 (h0 = 0; gates i, o, g; K=20) --------
                for j in range(BLK):
                    cA = j * 2 * W
                    G = pgp.tile([128, 2048], F32, tag="g")
                    for gn, r0 in (("i", 0), ("o", 32), ("g", 64)):
                        lhs = wk0s[r0:r0 + KX, 0:64]
                        s = SLOT[gn]
                        nc.tensor.matmul(G[0:64, s:s + W], lhs,
                                         xt0b[r0:r0 + KX, cA:cA + W],
                                         start=True, stop=True,
                                         tile_position=(r0, 0),
                                         skip_group_check=True)
                        nc.tensor.matmul(G[64:128, s:s + W], lhs,
                                         xt0b[r0:r0 + KX, cA + W:cA + 2 * W],
                                         start=True, stop=True,
                                         tile_position=(r0, 64),
                                         skip_group_check=True)
                    sg = sgp.tile([128, 2048], BF16, tag="sg")
                    nc.scalar.activation(sg[:, 0:3 * W], G[:, 0:3 * W],
                                         AF.Sigmoid)
                    c1 = cwp.tile([128, W], BF16, tag="c")
                    nc.vector._custom_dve(ig_op, out=c1[:], in0=sg[:, 0:W],
                                          in1=sg[:, 2 * W:3 * W])
                    nc.vector._custom_dve(th5_op, out=H1[:, j * W:(j + 1) * W],
                                          in0=c1[:], in1=sg[:, W:2 * W],
                                          s0=T5C3, s1=T5C5)
                    cs[j] = c1
                    if j == BLK // 2 - 1:
                        shift_h(H1, s1b, 0)
                shift_h(H1, s1b, 1)

                # ---- phase 2: step 2 (4 gates; K=84) ----------------------
                for j in range(BLK):
                    cA = j * 2 * W
                    G = pgp.tile([128, 2048], F32, tag="g")
                    gate_mms(wks, KS, s1b, cA, G, ("i", "o", "g", "f"))
                    sg = sgp.tile([128, 2048], BF16, tag="sg")
                    nc.scalar.activation(sg[:], G[:], AF.Sigmoid)
                    u2 = uvp.tile([128, W], BF16, tag="u")
                    nc.vector._custom_dve(ig_op, out=u2[:], in0=sg[:, 0:W],
                                          in1=sg[:, 2 * W:3 * W])
                    v2 = uvp.tile([128, W], BF16, tag="v")
                    nc.gpsimd.tensor_mul(v2[:], sg[:, 3 * W:4 * W], cs[j][:])
                    c2 = cwp.tile([128, W], BF16, tag="c")
                    nc.gpsimd.tensor_add(c2[:], u2[:], v2[:])
                    nc.vector._custom_dve(th5_op, out=H2[:, j * W:(j + 1) * W],
                                          in0=c2[:], in1=sg[:, W:2 * W],
                                          s0=T5C3, s1=T5C5)
                    cs[j] = c2
                    if j == BLK // 2 - 1:
                        shift_h(H2, s2b, 0)
                shift_h(H2, s2b, 1)

                # ---- phase 3: step 3 + MLP + value ------------------------
                for j in range(BLK):
                    cA = j * 2 * W
                    G = pgp.tile([128, 2048], F32, tag="g")
                    gate_mms(wks, KS, s2b, cA, G, ("i", "o", "g", "f"))
                    sg = sgp.tile([128, 2048], BF16, tag="sg")
                    nc.scalar.activation(sg[:], G[:], AF.Sigmoid)
                    u3 = uvp.tile([128, W], BF16, tag="u")
                    nc.vector._custom_dve(ig_op, out=u3[:], in0=sg[:, 0:W],
                                          in1=sg[:, 2 * W:3 * W])
                    v3 = uvp.tile([128, W], BF16, tag="v")
                    nc.gpsimd.tensor_mul(v3[:], sg[:, 3 * W:4 * W], cs[j][:])
                    c3 = cwp.tile([128, W], BF16, tag="c")
                    nc.gpsimd.tensor_add(c3[:], u3[:], v3[:])
                    th3 = uvp.tile([128, W], BF16, tag="th")
                    nc.vector._custom_dve(t7_op, out=th3[:], in0=c3[:],
                                          in1=c3t[:], s0=T7C[0], s1=T7C[1],
                                          imm2=T7C[2])
                    h3 = h3p.tile([128, W], BF16, tag="h3")
                    nc.vector.tensor_mul(h3[:], sg[:, W:2 * W], th3[:])

                    zp = pgp.tile([128, 2048], F32, tag="g")
                    nc.tensor.matmul(zp[0:33, 0:W], w1r[0:64, 0:33],
                                     h3[0:64, :], start=True, stop=True,
                                     tile_position=(0, 0),
                                     skip_group_check=True)
                    nc.tensor.matmul(zp[64:97, 0:W], w1r[64:128, 0:33],
                                     h3[64:128, :], start=True, stop=True,
                                     tile_position=(64, 64),
                                     skip_group_check=True)
                    zr = zrp.tile([97, W], BF16, tag="zr")
                    nc.vector.tensor_scalar(zr[:], zp[0:97, 0:W], b1r[:], 0.0,
                                            ALU.add, ALU.max)

                    vb = pgp.tile([128, 2048], F32, tag="g")
                    nc.tensor.matmul(vb[0:1, 0:W], w2r[0:33, 0:1],
                                     zr[0:33, :], start=True, stop=True,
                                     tile_position=(0, 0),
                                     skip_group_check=True)
                    nc.tensor.matmul(vb[32:33, 0:W], w2r[64:97, 0:1],
                                     zr[64:97, :], start=True, stop=True,
                                     tile_position=(64, 32),
                                     skip_group_check=True)
                    nc.scalar.copy(vo[0:33, j * W:(j + 1) * W], vb[0:33, 0:W])

                nc.sync.dma_start(
                    out_d[0:1, blk * BLK:(blk + 1) * BLK, :].rearrange(
                        "p a b -> p (a b)"), vo[0:1, :])
                nc.sync.dma_start(
                    out_d[1:2, blk * BLK:(blk + 1) * BLK, :].rearrange(
                        "p a b -> p (a b)"), vo[32:33, :])

    nc.compile()
    return nc


def _host_prep(state_seq, W_ih, W_hh, b_ih, b_hh, W1, b1, W2, b2):
    """Build per-core input maps (host-side layout prep only)."""
    bf = ml_dtypes.bfloat16
    B = state_seq.shape[0]
    b = (b_ih.astype(np.float64) + b_hh.astype(np.float64))  # [256]

    # gate column order [i o g f]; pytorch rows: i 0:64, f 64:128, g 128:192,
    # o 192:256.  g-gate scaled by 2 (tanh(g) = 2*sigmoid(2g) - 1).
    perm = np.concatenate([np.arange(0, 64), np.arange(192, 256),
                           np.arange(128, 192), np.arange(64, 128)])
    scale = np.ones(256)
    scale[128:192] = 2.0  # g block in [i o g f] order
    Wih_p = W_ih[perm, :].astype(np.float64) * scale[:, None]
    Whh_p = W_hh[perm, :].astype(np.float64) * scale[:, None]
    b_p = b[perm] * scale

    b_hi = b_p.astype(bf).astype(np.float64)
    b_lo = (b_p - b_hi).astype(bf).astype(np.float64)

    wks = np.zeros((KS, 256), np.float64)
    wks[0:HIDDEN, :] = Whh_p.T
    wks[HIDDEN:HIDDEN + STATE_DIM, :] = Wih_p.T
    wks[HIDDEN + STATE_DIM, :] = b_hi
    wks[HIDDEN + STATE_DIM + 1, :] = b_lo

    # step-1 weights: per-gate [20,64] blocks at partition rows 0/32/64
    # for gates i, o, g (column blocks 0:64, 64:128, 128:192 of packed)
    wk0s = np.zeros((KS, 64), np.float64)
    for gi, r0 in ((0, 0), (1, 32), (2, 64)):  # i, o, g
        cols = slice(gi * 64, (gi + 1) * 64)
        wk0s[r0:r0 + STATE_DIM, :] = Wih_p.T[:, cols]
        wk0s[r0 + STATE_DIM, :] = b_hi[cols]
        wk0s[r0 + STATE_DIM + 1, :] = b_lo[cols]

    # W1 augmented with a 33rd zero-row (z33 = relu(0 + 1) = 1), b2 -> W2'
    w1r = np.zeros((128, 33), np.float64)
    w1r[0:64, 0:32] = W1.astype(np.float64).T
    w1r[64:128, 0:32] = W1.astype(np.float64).T
    w2r = np.zeros((97, 1), np.float64)
    w2r[0:32, 0] = W2[0].astype(np.float64)
    w2r[32, 0] = float(b2[0])
    w2r[64:96, 0] = W2[0].astype(np.float64)
    w2r[96, 0] = float(b2[0])
    b1r = np.zeros((97, 1), np.float32)
    b1r[0:32, 0] = b1
    b1r[32, 0] = 1.0
    b1r[64:96, 0] = b1
    b1r[96, 0] = 1.0

    # xt arrays: [KX, B] bf16: rows 0:18 = x_t.T, rows 18,19 = ones
    xts = []
    for t in range(SEQ_LEN):
        a = np.ones((KX, B), np.float32)
        a[0:STATE_DIM, :] = state_seq[:, t, :].T
        xts.append(a.astype(bf))

    shared = {
        "wk0s": wk0s.astype(bf), "wks": wks.astype(bf),
        "w1r": w1r.astype(bf), "w2r": w2r.astype(bf), "b1r": b1r,
    }
    in_maps = []
    for cc in range(NCORES):
        lo, hi = cc * BCORE, (cc + 1) * BCORE
        m = dict(shared)
        m["xt0"] = np.ascontiguousarray(xts[0][:, lo:hi])
        m["xt1"] = np.ascontiguousarray(xts[1][:, lo:hi])
        m["xt2"] = np.ascontiguousarray(xts[2][:, lo:hi])
        in_maps.append(m)
    return in_maps


_cached = {}


def kernel(**inputs) -> np.ndarray:
    if "nc" not in _cached:
        _cached["nc"] = build_bass()
    nc = _cached["nc"]
    in_maps = _host_prep(**inputs)
    trace = bool(int(os.environ.get("K_TRACE", "0")))
    res = bass_utils.run_bass_kernel_spmd(nc, in_maps, core_ids=list(range(NCORES)),
                                          trace=trace)
    outs = [np.asarray(r["out"]).transpose(1, 0, 2).reshape(-1)
            for r in res.results]
    _cached["last_results"] = res
    return np.concatenate(outs).astype(np.float32)
